# revision 10
# baseline (speedup 1.0000x reference)
"""Trainium2 Bass kernel for nn_Conv2dModulation.

Math (per sample b):
    w0 = weight * c,  c = (cin*3*3)^-0.5
    w1[o,i,kh,kw] = w0[o,i,kh,kw] * y[b,i]
    d[o] = rsqrt(sum_{i,kh,kw} w1^2 + eps)
    out[b] = conv2d_SAME(X[b], w1 * d)

Device strategy (per core, 2 samples):
  - Modulation/demodulation on device in fp32, stored bf16.  The
    per-(s,o) demod factor is broadcast across partitions with a tiny
    indicator matmul (no DRAM roundtrip); Kaiming const folded into the
    indicator.  Prologue inputs are host-packed into two contiguous
    tensors (w2, aux) so only 2 small DMAs precede the weight chain,
    and the first X chunk's DMA is issued ahead of them (split in two
    so the first row-groups can start early).
  - X is zero-padded on host to [H+2, W+2] so each 32-row chunk is ONE
    fully contiguous DMA (17.5KB/partition) and no memsets are needed.
  - Conv = 9 shifted matmuls (taps) accumulating into PSUM.  All four
    64x64 PE-array quadrants run concurrently: rows = sample (s),
    cols = output-row pair (q), via tile_position=(s*64, q*64).  The
    (s,q) matmul writes PSUM bank_s partitions [q*64:(q+1)*64].
  - PSUM (fp32) evacuated to SBUF bf16 on DVE (s=0) / ACT (s=1) with a
    partition remap (q,o)->(s,o); one batched DMA per 2 row-groups
    (8 output rows) back to HBM in bf16 (host converts to fp32).
"""

import numpy as np
import ml_dtypes

import concourse.bass as bass
import concourse.tile as tile
from concourse import bacc, mybir
from concourse.bass_utils import run_bass_kernel_spmd

F32 = mybir.dt.float32
BF16 = mybir.dt.bfloat16
NPBF16 = ml_dtypes.bfloat16

B, C, H, W, KS = 16, 64, 256, 256, 3
NCORES = 8
SPC = B // NCORES          # samples per core = 2
WP = W + 2                 # padded row width
HP = H + 2                 # padded column height
EPS = 1e-8
CKAIMING = float((C * KS * KS) ** -0.5)

R = 32                     # output rows per chunk
NCHUNK = H // R

XT_BUFS = 3


def build_program(nc):
    Xl = nc.dram_tensor("Xl", [SPC * C, HP, WP], BF16, kind="ExternalInput")
    # w2[(s,i), (t,o)] = wT[t,i,o] pre-replicated on both halves (host)
    w2 = nc.dram_tensor("w2", [2 * C, KS * KS * C], F32, kind="ExternalInput")
    # aux: col0 = y[(s,i)]; col1..2 = y.T (partitions 0..63);
    #      col4..131 = indicator c*(s==s2) (partitions 0..1)
    aux = nc.dram_tensor("aux", [2 * C, 132], F32, kind="ExternalInput")
    out = nc.dram_tensor("out", [SPC * C, H, W], BF16, kind="ExternalOutput")

    with tile.TileContext(nc) as tc:
        with (
            tc.tile_pool(name="wpool", bufs=1) as wpool,
            tc.tile_pool(name="xpool", bufs=XT_BUFS) as xpool,
            tc.tile_pool(name="opool", bufs=4) as opool,
            tc.tile_pool(name="psA", bufs=4, space="PSUM") as psA,
            tc.tile_pool(name="psB", bufs=4, space="PSUM") as psB,
        ):
            # Sqrt activation-table preload (overlaps the DMAs below)
            eps_t = wpool.tile([SPC, 1], F32)
            nc.gpsimd.memset(eps_t[:, :], EPS)
            warm = wpool.tile([SPC, 1], F32)
            nc.scalar.activation(warm[:, :], eps_t[:, :],
                                 mybir.ActivationFunctionType.Sqrt,
                                 bias=eps_t[:, :], scale=1.0)

            # first X chunk, split so early row-groups unblock sooner
            xt0 = xpool.tile([2 * C, (R + 2) * WP], BF16)
            xt0v = xt0[:, :].rearrange("p (r w) -> p r w", w=WP)
            nc.sync.dma_start(xt0v[:, 0:18, :], Xl.ap()[:, 0:18, :])
            nc.sync.dma_start(xt0v[:, 18:R + 2, :], Xl.ap()[:, 18:R + 2, :])

            w2s = wpool.tile([2 * C, KS * KS * C], F32)
            nc.sync.dma_start(w2s[:, :], w2.ap()[:, :])
            auxs = wpool.tile([2 * C, 132], F32)
            nc.sync.dma_start(auxs[:, :], aux.ap()[:, :])

            # wsq / s_acc -> dpre -> d chain
            wsq = wpool.tile([C, KS * KS * C], F32)
            nc.vector.tensor_mul(wsq[:, :], w2s[0:C, :], w2s[0:C, :])
            s_acc = wpool.tile([C, C], F32)
            nc.vector.tensor_reduce(
                s_acc[:, :],
                wsq[:, :].rearrange("p (t o) -> p o t", t=KS * KS),
                mybir.AxisListType.X, mybir.AluOpType.add)
            ysq = wpool.tile([C, SPC], F32)
            nc.vector.tensor_mul(ysq[:, :], auxs[0:C, 1:3], auxs[0:C, 1:3])

            # wy = w2 * y  (overlaps the d chain)
            wy = wpool.tile([2 * C, KS * KS * C], F32)
            nc.vector.tensor_scalar(wy[:, :], w2s[:, :], auxs[:, 0:1],
                                    None, mybir.AluOpType.mult)

            # dpre[s, o] = sum_i ysq[i,s] * S_T[i,o]
            dpre = psA.tile([SPC, C], F32, name="dpre", tag="p00")
            nc.tensor.matmul(dpre[:, :], ysq[:, :], s_acc[:, :],
                             start=True, stop=True)
            dsq = wpool.tile([SPC, C], F32)
            nc.scalar.activation(dsq[:, :], dpre[:, :],
                                 mybir.ActivationFunctionType.Sqrt,
                                 bias=eps_t[:, :], scale=CKAIMING * CKAIMING)
            drow = wpool.tile([SPC, C], F32)
            nc.vector.reciprocal(drow[:, :], dsq[:, :])
            # dfull[(s,i), o] = c * drow[s, o]  via indicator matmul
            dfull = psB.tile([2 * C, C], F32, name="dfull", tag="p10")
            nc.tensor.matmul(dfull[:, :], auxs[0:SPC, 4:132], drow[:, :],
                             start=True, stop=True)

            # wmod16[(s,i), t*64+o] = wy * c*d  (DVE reads PSUM operand)
            wmod16 = wpool.tile([2 * C, KS * KS * C], BF16)
            nc.vector.tensor_tensor(
                wmod16[:, :].rearrange("p (t o) -> p t o", t=KS * KS),
                wy[:, :].rearrange("p (t o) -> p t o", t=KS * KS),
                dfull[:, :].unsqueeze(1).broadcast_to([2 * C, KS * KS, C]),
                mybir.AluOpType.mult,
            )

            # ---- conv main loop ----
            for ci in range(NCHUNK):
                r0 = ci * R
                if ci == 0:
                    xt3 = xt0v
                else:
                    xt = xpool.tile([2 * C, (R + 2) * WP], BF16)
                    xt3 = xt[:, :].rearrange("p (r w) -> p r w", w=WP)
                    nc.sync.dma_start(xt3[:, :, :],
                                      Xl.ap()[:, r0:r0 + R + 2, :])

                for rbp in range(R // 8):          # pairs of row-groups
                    last_pair = (ci == NCHUNK - 1 and rbp == R // 8 - 1)
                    ostage = opool.tile([2 * C, 2 * 1024], BF16)
                    for g in range(2):
                        rb = rbp * 2 + g
                        ps = [
                            psA.tile([2 * C, 512], F32,
                                     name=f"ps0_{ci}_{rb}", tag="p00"),
                            psB.tile([2 * C, 512], F32,
                                     name=f"ps1_{ci}_{rb}", tag="p10"),
                        ]
                        for t in range(KS * KS):
                            dh, dw = t // KS - 1, t % KS - 1
                            for s in range(SPC):
                                lhsT = wmod16[s * C:(s + 1) * C,
                                              t * C:(t + 1) * C]
                                for q in range(2):
                                    lr = rb * 4 + 2 * q + dh + 1
                                    co = dw + 1
                                    rhs = xt3[s * C:(s + 1) * C,
                                              lr:lr + 2, co:co + W]
                                    nc.tensor.matmul(
                                        ps[s][q * C:(q + 1) * C, :],
                                        lhsT,
                                        rhs,
                                        start=(t == 0),
                                        stop=(t == KS * KS - 1),
                                        tile_position=(s * C, q * C),
                                        skip_group_check=True,
                                    )
                        # evacuate PSUM -> SBUF bf16 with partition remap
                        # (q,o) -> (s,o); DVE: s=0, ACT: s=1
                        for q in range(2):
                            nc.vector.tensor_copy(
                                ostage[0:C,
                                       g * 1024 + q * 512:
                                       g * 1024 + (q + 1) * 512],
                                ps[0][q * C:(q + 1) * C, :])
                            nc.scalar.copy(
                                ostage[C:2 * C,
                                       g * 1024 + q * 512:
                                       g * 1024 + (q + 1) * 512],
                                ps[1][q * C:(q + 1) * C, :])
                        if last_pair:
                            # flush per row-group so the tail DMA is short
                            rr = r0 + rb * 4
                            dstap = out.ap()[:, rr:rr + 4, :].rearrange(
                                "so (q t) w -> so q (t w)", q=2)
                            nc.sync.dma_start(
                                dstap,
                                ostage[:, g * 1024:(g + 1) * 1024]
                                .rearrange("p (q tw) -> p q tw", q=2))
                    if not last_pair:
                        rr = r0 + rbp * 8
                        dstap = out.ap()[:, rr:rr + 8, :].rearrange(
                            "so (gq t) w -> so gq (t w)", gq=4)
                        nc.sync.dma_start(
                            dstap,
                            ostage[:, :].rearrange(
                                "p (gq tw) -> p gq tw", gq=4))

    return nc


_CACHED = {}


def _get_compiled():
    if "nc" not in _CACHED:
        nc = bacc.Bacc("TRN2", debug=False)
        build_program(nc)
        nc.compile()
        _CACHED["nc"] = nc
    return _CACHED["nc"]


def make_in_maps(X, y, weight):
    X = np.ascontiguousarray(X, dtype=np.float32)
    y = np.ascontiguousarray(y, dtype=np.float32)
    weight = np.ascontiguousarray(weight, dtype=np.float32)
    Xp = np.zeros((B, C, HP, WP), dtype=NPBF16)
    Xp[:, :, 1:H + 1, 1:W + 1] = X.astype(NPBF16)
    # w2[(s,i), (t,o)] = weight[o,i,kh,kw] transposed, replicated halves
    wT = weight.transpose(2, 3, 1, 0).reshape(KS * KS * C, C)  # [(t,i), o]
    wTi = wT.reshape(KS * KS, C, C).transpose(1, 0, 2).reshape(
        C, KS * KS * C)                                        # [i, (t,o)]
    w2 = np.ascontiguousarray(np.concatenate([wTi, wTi], axis=0))
    in_maps = []
    for c in range(NCORES):
        xs = Xp[c * SPC:(c + 1) * SPC]
        ys = y[c * SPC:(c + 1) * SPC]
        auxm = np.zeros((2 * C, 132), dtype=np.float32)
        auxm[:, 0] = ys.reshape(2 * C)
        auxm[0:C, 1:3] = ys.T
        for s in range(SPC):
            auxm[s, 4 + s * C:4 + (s + 1) * C] = CKAIMING
        in_maps.append({
            "Xl": np.ascontiguousarray(xs.reshape(SPC * C, HP, WP)),
            "w2": w2,
            "aux": auxm,
        })
    return in_maps


def kernel(X, y, weight):
    nc = _get_compiled()
    in_maps = make_in_maps(X, y, weight)
    res = run_bass_kernel_spmd(nc, in_maps, core_ids=list(range(NCORES)))
    outs = [res.results[c]["out"].astype(np.float32).reshape(SPC, C, H, W)
            for c in range(NCORES)]
    return np.concatenate(outs, axis=0)


# revision 11
# speedup vs baseline: 1.0374x; 1.0374x over previous
"""Trainium2 Bass kernel for nn_Conv2dModulation.

Math (per sample b):
    w0 = weight * c,  c = (cin*3*3)^-0.5
    w1[o,i,kh,kw] = w0[o,i,kh,kw] * y[b,i]
    d[o] = rsqrt(sum_{i,kh,kw} w1^2 + eps)
    out[b] = conv2d_SAME(X[b], w1 * d)

Key restructure: d scales each output filter uniformly, so
    out[s,o] = d[s,o] * conv2d(X[s], w0 * y[s])
and d is applied at PSUM evacuation as a per-partition scalar multiply.
The conv therefore only waits on w2 (base weight) and one DVE
tensor_scalar (wmod16 = w2 * c*y, bf16); the whole d chain
(wsq -> s_acc -> dpreT -> sqrt -> recip -> eye-matmul broadcast) runs
in the background before the first evacuation needs it.

Device strategy (per core, 2 samples):
  - X zero-padded on host to [H+2, W+2]; each 32-row chunk is one fully
    contiguous DMA (chunk 0 split in three so early row-groups start
    ~13us in).  Small prologue tensors (w2, aux) are host-packed and
    DMA'd first.
  - Conv = 9 shifted matmuls (taps) accumulating into PSUM.  All four
    64x64 PE-array quadrants run concurrently: rows = sample (s),
    cols = output-row pair (q), via tile_position=(s*64, q*64).  The
    (s,q) matmul writes PSUM bank_s partitions [q*64:(q+1)*64].
  - PSUM (fp32) evacuated to SBUF bf16 with the demod multiply fused:
    DVE tensor_scalar (s=0) / ACT activation-Copy-with-scale (s=1),
    partition remap (q,o)->(s,o); one batched DMA per 2 row-groups
    (8 output rows) back to HBM in bf16 (host converts to fp32).
"""

import numpy as np
import ml_dtypes

import concourse.bass as bass
import concourse.tile as tile
from concourse import bacc, mybir
from concourse.bass_utils import run_bass_kernel_spmd

F32 = mybir.dt.float32
BF16 = mybir.dt.bfloat16
NPBF16 = ml_dtypes.bfloat16

B, C, H, W, KS = 16, 64, 256, 256, 3
NCORES = 8
SPC = B // NCORES          # samples per core = 2
WP = W + 2                 # padded row width
HP = H + 2                 # padded column height
EPS = 1e-8
CKAIMING = float((C * KS * KS) ** -0.5)

R = 32                     # output rows per chunk
NCHUNK = H // R

XT_BUFS = 3


def build_program(nc):
    Xl = nc.dram_tensor("Xl", [SPC * C, HP, WP], BF16, kind="ExternalInput")
    # w2[(s,i), (t,o)] = wT[t,i,o] pre-replicated on both halves (host)
    w2 = nc.dram_tensor("w2", [2 * C, KS * KS * C], F32, kind="ExternalInput")
    # aux: col0 = c*y[(s,i)]; col1..2 = y.T (partitions 0..63);
    #      col4..131 = eye broadcast I[o', h*64+o]=(o==o') (partitions 0..63)
    aux = nc.dram_tensor("aux", [2 * C, 132], F32, kind="ExternalInput")
    out = nc.dram_tensor("out", [SPC * C, H, W], BF16, kind="ExternalOutput")

    with tile.TileContext(nc) as tc:
        with (
            tc.tile_pool(name="wpool", bufs=1) as wpool,
            tc.tile_pool(name="xpool", bufs=XT_BUFS) as xpool,
            tc.tile_pool(name="opool", bufs=4) as opool,
            tc.tile_pool(name="psA", bufs=4, space="PSUM") as psA,
            tc.tile_pool(name="psB", bufs=4, space="PSUM") as psB,
        ):
            eps64 = wpool.tile([C, 1], F32)
            nc.gpsimd.memset(eps64[:, :], EPS)

            # small gating DMAs first, then X chunk 0 in three slices
            w2s = wpool.tile([2 * C, KS * KS * C], F32)
            nc.sync.dma_start(w2s[:, :], w2.ap()[:, :])
            auxs = wpool.tile([2 * C, 132], F32)
            nc.sync.dma_start(auxs[:, :], aux.ap()[:, :])

            xt0 = xpool.tile([2 * C, (R + 2) * WP], BF16)
            xt0v = xt0[:, :].rearrange("p (r w) -> p r w", w=WP)
            for lo, hi in ((0, 10), (10, 22), (22, R + 2)):
                nc.sync.dma_start(xt0v[:, lo:hi, :], Xl.ap()[:, lo:hi, :])

            # conv weights: wmod16 = w2 * (c*y)   (gates the conv)
            wmod16 = wpool.tile([2 * C, KS * KS * C], BF16)
            nc.vector.tensor_scalar(wmod16[:, :], w2s[:, :], auxs[:, 0:1],
                                    None, mybir.AluOpType.mult)

            # ---- background demod chain ----
            wsq = wpool.tile([C, KS * KS * C], F32)
            nc.vector.tensor_mul(wsq[:, :], w2s[0:C, :], w2s[0:C, :])
            s_acc = wpool.tile([C, C], F32)
            nc.vector.tensor_reduce(
                s_acc[:, :],
                wsq[:, :].rearrange("p (t o) -> p o t", t=KS * KS),
                mybir.AxisListType.X, mybir.AluOpType.add)
            ysq = wpool.tile([C, SPC], F32)
            nc.vector.tensor_mul(ysq[:, :], auxs[0:C, 1:3], auxs[0:C, 1:3])

            # dpreT[o, s] = sum_i S_T[i,o] * ysq[i,s]
            dpreT = psA.tile([C, SPC], F32, name="dpreT", tag="p00")
            nc.tensor.matmul(dpreT[:, :], s_acc[:, :], ysq[:, :],
                             start=True, stop=True)
            dsqT = wpool.tile([C, SPC], F32)
            nc.scalar.activation(dsqT[:, :], dpreT[:, :],
                                 mybir.ActivationFunctionType.Sqrt,
                                 bias=eps64[:, :], scale=CKAIMING * CKAIMING)
            drowT = wpool.tile([C, SPC], F32)
            nc.vector.reciprocal(drowT[:, :], dsqT[:, :])
            # dall[(h,o), s] = drowT[o, s] on both partition halves
            dallp = psB.tile([2 * C, SPC], F32, name="dall", tag="p10")
            nc.tensor.matmul(dallp[:, :], auxs[0:C, 4:132], drowT[:, :],
                             start=True, stop=True)
            dalls = wpool.tile([2 * C, SPC], F32)
            nc.vector.tensor_copy(dalls[:, :], dallp[:, :])

            # ---- conv main loop ----
            for ci in range(NCHUNK):
                r0 = ci * R
                if ci == 0:
                    xt3 = xt0v
                else:
                    xt = xpool.tile([2 * C, (R + 2) * WP], BF16)
                    xt3 = xt[:, :].rearrange("p (r w) -> p r w", w=WP)
                    nc.sync.dma_start(xt3[:, :, :],
                                      Xl.ap()[:, r0:r0 + R + 2, :])

                for rbp in range(R // 8):          # pairs of row-groups
                    last_pair = (ci == NCHUNK - 1 and rbp == R // 8 - 1)
                    ostage = opool.tile([2 * C, 2 * 1024], BF16)
                    for g in range(2):
                        rb = rbp * 2 + g
                        ps = [
                            psA.tile([2 * C, 512], F32,
                                     name=f"ps0_{ci}_{rb}", tag="p00"),
                            psB.tile([2 * C, 512], F32,
                                     name=f"ps1_{ci}_{rb}", tag="p10"),
                        ]
                        for t in range(KS * KS):
                            dh, dw = t // KS - 1, t % KS - 1
                            for s in range(SPC):
                                lhsT = wmod16[s * C:(s + 1) * C,
                                              t * C:(t + 1) * C]
                                for q in range(2):
                                    lr = rb * 4 + 2 * q + dh + 1
                                    co = dw + 1
                                    rhs = xt3[s * C:(s + 1) * C,
                                              lr:lr + 2, co:co + W]
                                    nc.tensor.matmul(
                                        ps[s][q * C:(q + 1) * C, :],
                                        lhsT,
                                        rhs,
                                        start=(t == 0),
                                        stop=(t == KS * KS - 1),
                                        tile_position=(s * C, q * C),
                                        skip_group_check=True,
                                    )
                        # evacuate PSUM -> SBUF bf16, fusing the demod
                        # multiply; partition remap (q,o) -> (s,o);
                        # DVE: s=0, ACT: s=1
                        for q in range(2):
                            nc.vector.tensor_scalar(
                                ostage[0:C,
                                       g * 1024 + q * 512:
                                       g * 1024 + (q + 1) * 512],
                                ps[0][q * C:(q + 1) * C, :],
                                dalls[0:C, 0:1],
                                None, mybir.AluOpType.mult)
                            nc.scalar.activation(
                                ostage[C:2 * C,
                                       g * 1024 + q * 512:
                                       g * 1024 + (q + 1) * 512],
                                ps[1][q * C:(q + 1) * C, :],
                                mybir.ActivationFunctionType.Copy,
                                scale=dalls[C:2 * C, 1:2])
                        if last_pair:
                            # flush per row-group so the tail DMA is short
                            rr = r0 + rb * 4
                            dstap = out.ap()[:, rr:rr + 4, :].rearrange(
                                "so (q t) w -> so q (t w)", q=2)
                            nc.sync.dma_start(
                                dstap,
                                ostage[:, g * 1024:(g + 1) * 1024]
                                .rearrange("p (q tw) -> p q tw", q=2))
                    if not last_pair:
                        rr = r0 + rbp * 8
                        dstap = out.ap()[:, rr:rr + 8, :].rearrange(
                            "so (gq t) w -> so gq (t w)", gq=4)
                        nc.sync.dma_start(
                            dstap,
                            ostage[:, :].rearrange(
                                "p (gq tw) -> p gq tw", gq=4))

    return nc


_CACHED = {}


def _get_compiled():
    if "nc" not in _CACHED:
        nc = bacc.Bacc("TRN2", debug=False)
        build_program(nc)
        nc.compile()
        _CACHED["nc"] = nc
    return _CACHED["nc"]


def make_in_maps(X, y, weight):
    X = np.ascontiguousarray(X, dtype=np.float32)
    y = np.ascontiguousarray(y, dtype=np.float32)
    weight = np.ascontiguousarray(weight, dtype=np.float32)
    Xp = np.zeros((B, C, HP, WP), dtype=NPBF16)
    Xp[:, :, 1:H + 1, 1:W + 1] = X.astype(NPBF16)
    # w2[(s,i), (t,o)] = weight[o,i,kh,kw] transposed, replicated halves
    wTi = weight.transpose(2, 3, 1, 0).reshape(KS * KS, C, C)  # [t, i, o]
    wTi = wTi.transpose(1, 0, 2).reshape(C, KS * KS * C)       # [i, (t,o)]
    w2 = np.ascontiguousarray(np.concatenate([wTi, wTi], axis=0))
    eye2 = np.tile(np.eye(C, dtype=np.float32), (1, SPC))      # [o', (h,o)]
    in_maps = []
    for c in range(NCORES):
        xs = Xp[c * SPC:(c + 1) * SPC]
        ys = y[c * SPC:(c + 1) * SPC]
        auxm = np.zeros((2 * C, 132), dtype=np.float32)
        auxm[:, 0] = CKAIMING * ys.reshape(2 * C)
        auxm[0:C, 1:3] = ys.T
        auxm[0:C, 4:132] = eye2
        in_maps.append({
            "Xl": np.ascontiguousarray(xs.reshape(SPC * C, HP, WP)),
            "w2": w2,
            "aux": auxm,
        })
    return in_maps


def kernel(X, y, weight):
    nc = _get_compiled()
    in_maps = make_in_maps(X, y, weight)
    res = run_bass_kernel_spmd(nc, in_maps, core_ids=list(range(NCORES)))
    outs = [res.results[c]["out"].astype(np.float32).reshape(SPC, C, H, W)
            for c in range(NCORES)]
    return np.concatenate(outs, axis=0)


# revision 15
# speedup vs baseline: 1.0539x; 1.0159x over previous
"""Trainium2 Bass kernel for nn_Conv2dModulation.

Math (per sample b):
    w0 = weight * c,  c = (cin*3*3)^-0.5
    w1[o,i,kh,kw] = w0[o,i,kh,kw] * y[b,i]
    d[o] = rsqrt(sum_{i,kh,kw} w1^2 + eps)
    out[b] = conv2d_SAME(X[b], w1 * d)

Key restructure: d scales each output filter uniformly, so
    out[s,o] = d[s,o] * conv2d(X[s], w0 * y[s])
and d is applied at PSUM evacuation as a per-partition scalar multiply.
The conv therefore only waits on w2 (base weight) and one DVE
tensor_scalar (wmod16 = w2 * c*y, bf16); the whole d chain
(wsq -> s_acc -> dpreT -> sqrt -> recip -> eye-matmul broadcast) runs
in the background before the first evacuation needs it.

Device strategy (per core, 2 samples):
  - X zero-padded on host to [H+2, W+2]; each 32-row chunk is one fully
    contiguous DMA (chunk 0 split in three so early row-groups start
    ~13us in).  Small prologue tensors (w2, aux) are host-packed and
    DMA'd first.
  - Conv = 9 shifted matmuls (taps) accumulating into PSUM.  All four
    64x64 PE-array quadrants run concurrently: rows = sample (s),
    cols = output-row pair (q), via tile_position=(s*64, q*64).  The
    (s,q) matmul writes PSUM bank_s partitions [q*64:(q+1)*64].
  - PSUM (fp32) evacuated to SBUF bf16 with the demod multiply fused:
    DVE tensor_scalar (s=0) / ACT activation-Copy-with-scale (s=1),
    partition remap (q,o)->(s,o); one batched DMA per 2 row-groups
    (8 output rows) back to HBM in bf16 (host converts to fp32).
"""

import numpy as np
import ml_dtypes

import concourse.bass as bass
import concourse.tile as tile
from concourse import bacc, mybir
from concourse.bass_utils import run_bass_kernel_spmd

F32 = mybir.dt.float32
BF16 = mybir.dt.bfloat16
NPBF16 = ml_dtypes.bfloat16

B, C, H, W, KS = 16, 64, 256, 256, 3
NCORES = 8
SPC = B // NCORES          # samples per core = 2
WP = W + 2                 # padded row width
HP = H + 2                 # padded column height
EPS = 1e-8
CKAIMING = float((C * KS * KS) ** -0.5)

R = 32                     # output rows per chunk
NCHUNK = H // R

XT_BUFS = 3


def build_program(nc):
    Xl = nc.dram_tensor("Xl", [SPC * C, HP, WP], BF16, kind="ExternalInput")
    # w2x packs, per partition (s,i):
    #   col 0..575   = w2[(s,i), (t,o)] = wT[t,i,o] replicated on halves
    #   col 576      = c*y[(s,i)]
    #   col 577..578 = y.T (partitions 0..63)
    #   col 580..707 = eye broadcast I[o', h*64+o]=(o==o') (partitions 0..63)
    NW = KS * KS * C
    w2x = nc.dram_tensor("w2x", [2 * C, NW + 132], F32, kind="ExternalInput")
    out = nc.dram_tensor("out", [SPC * C, H, W], BF16, kind="ExternalOutput")

    with tile.TileContext(nc) as tc:
        with (
            tc.tile_pool(name="wpool", bufs=1) as wpool,
            tc.tile_pool(name="xpool", bufs=XT_BUFS) as xpool,
            tc.tile_pool(name="opool", bufs=4) as opool,
            tc.tile_pool(name="psA", bufs=4, space="PSUM") as psA,
            tc.tile_pool(name="psB", bufs=4, space="PSUM") as psB,
        ):
            eps64 = wpool.tile([C, 1], F32)
            nc.gpsimd.memset(eps64[:, :], EPS)

            # one gating DMA (weights + y + eye), then X chunk 0 in
            # need-ordered slices
            w2a = wpool.tile([2 * C, NW + 132], F32)
            nc.sync.dma_start(w2a[:, :], w2x.ap()[:, :])

            xt0 = xpool.tile([2 * C, (R + 2) * WP], BF16)
            xt0v = xt0[:, :].rearrange("p (r w) -> p r w", w=WP)
            for lo, hi in ((0, 6), (6, 10), (10, 18), (18, R + 2)):
                nc.sync.dma_start(xt0v[:, lo:hi, :], Xl.ap()[:, lo:hi, :])

            # conv weights: wmod16 = w2 * (c*y)   (gates the conv)
            wmod16 = wpool.tile([2 * C, KS * KS * C], BF16)
            nc.vector.tensor_scalar(wmod16[:, :], w2a[:, 0:NW],
                                    w2a[:, NW:NW + 1],
                                    None, mybir.AluOpType.mult)

            # ---- background demod chain ----
            wsq = wpool.tile([C, KS * KS * C], F32)
            nc.vector.tensor_mul(wsq[:, :], w2a[0:C, 0:NW],
                                 w2a[0:C, 0:NW])
            s_acc = wpool.tile([C, C], F32)
            nc.vector.tensor_reduce(
                s_acc[:, :],
                wsq[:, :].rearrange("p (t o) -> p o t", t=KS * KS),
                mybir.AxisListType.X, mybir.AluOpType.add)
            ysq = wpool.tile([C, SPC], F32)
            nc.vector.tensor_mul(ysq[:, :], w2a[0:C, NW + 1:NW + 3],
                                 w2a[0:C, NW + 1:NW + 3])

            # dpreT[o, s] = sum_i S_T[i,o] * ysq[i,s]
            dpreT = psA.tile([C, SPC], F32, name="dpreT", tag="p00")
            nc.tensor.matmul(dpreT[:, :], s_acc[:, :], ysq[:, :],
                             start=True, stop=True)
            dsqT = wpool.tile([C, SPC], F32)
            nc.scalar.activation(dsqT[:, :], dpreT[:, :],
                                 mybir.ActivationFunctionType.Sqrt,
                                 bias=eps64[:, :], scale=CKAIMING * CKAIMING)
            drowT = wpool.tile([C, SPC], F32)
            nc.vector.reciprocal(drowT[:, :], dsqT[:, :])
            # dall[(h,o), s] = drowT[o, s] on both partition halves
            dallp = psB.tile([2 * C, SPC], F32, name="dall", tag="p10")
            nc.tensor.matmul(dallp[:, :], w2a[0:C, NW + 4:NW + 132],
                             drowT[:, :],
                             start=True, stop=True)
            dalls = wpool.tile([2 * C, SPC], F32)
            nc.vector.tensor_copy(dalls[:, :], dallp[:, :])

            # ---- conv main loop ----
            for ci in range(NCHUNK):
                r0 = ci * R
                if ci == 0:
                    xt3 = xt0v
                else:
                    xt = xpool.tile([2 * C, (R + 2) * WP], BF16)
                    xt3 = xt[:, :].rearrange("p (r w) -> p r w", w=WP)
                    nc.sync.dma_start(xt3[:, :, :],
                                      Xl.ap()[:, r0:r0 + R + 2, :])

                for rbp in range(R // 8):          # pairs of row-groups
                    last_pair = (ci == NCHUNK - 1 and rbp == R // 8 - 1)
                    ostage = opool.tile([2 * C, 2 * 1024], BF16)
                    for g in range(2):
                        rb = rbp * 2 + g
                        ps = [
                            psA.tile([2 * C, 512], F32,
                                     name=f"ps0_{ci}_{rb}", tag="p00"),
                            psB.tile([2 * C, 512], F32,
                                     name=f"ps1_{ci}_{rb}", tag="p10"),
                        ]
                        for t in range(KS * KS):
                            dh, dw = t // KS - 1, t % KS - 1
                            for s in range(SPC):
                                lhsT = wmod16[s * C:(s + 1) * C,
                                              t * C:(t + 1) * C]
                                for q in range(2):
                                    lr = rb * 4 + 2 * q + dh + 1
                                    co = dw + 1
                                    rhs = xt3[s * C:(s + 1) * C,
                                              lr:lr + 2, co:co + W]
                                    nc.tensor.matmul(
                                        ps[s][q * C:(q + 1) * C, :],
                                        lhsT,
                                        rhs,
                                        start=(t == 0),
                                        stop=(t == KS * KS - 1),
                                        tile_position=(s * C, q * C),
                                        skip_group_check=True,
                                    )
                        # evacuate PSUM -> SBUF bf16, fusing the demod
                        # multiply; partition remap (q,o) -> (s,o);
                        # DVE: s=0, ACT: s=1
                        for q in range(2):
                            nc.vector.tensor_scalar(
                                ostage[0:C,
                                       g * 1024 + q * 512:
                                       g * 1024 + (q + 1) * 512],
                                ps[0][q * C:(q + 1) * C, :],
                                dalls[0:C, 0:1],
                                None, mybir.AluOpType.mult)
                            nc.scalar.activation(
                                ostage[C:2 * C,
                                       g * 1024 + q * 512:
                                       g * 1024 + (q + 1) * 512],
                                ps[1][q * C:(q + 1) * C, :],
                                mybir.ActivationFunctionType.Copy,
                                scale=dalls[C:2 * C, 1:2])
                        if last_pair:
                            # flush per row-group so the tail DMA is short
                            rr = r0 + rb * 4
                            dstap = out.ap()[:, rr:rr + 4, :].rearrange(
                                "so (q t) w -> so q (t w)", q=2)
                            nc.sync.dma_start(
                                dstap,
                                ostage[:, g * 1024:(g + 1) * 1024]
                                .rearrange("p (q tw) -> p q tw", q=2))
                    if not last_pair:
                        rr = r0 + rbp * 8
                        dstap = out.ap()[:, rr:rr + 8, :].rearrange(
                            "so (gq t) w -> so gq (t w)", gq=4)
                        nc.sync.dma_start(
                            dstap,
                            ostage[:, :].rearrange(
                                "p (gq tw) -> p gq tw", gq=4))

    return nc


_CACHED = {}


def _get_compiled():
    if "nc" not in _CACHED:
        nc = bacc.Bacc("TRN2", debug=False)
        build_program(nc)
        nc.compile()
        _CACHED["nc"] = nc
    return _CACHED["nc"]


def make_in_maps(X, y, weight):
    X = np.ascontiguousarray(X, dtype=np.float32)
    y = np.ascontiguousarray(y, dtype=np.float32)
    weight = np.ascontiguousarray(weight, dtype=np.float32)
    Xp = np.zeros((B, C, HP, WP), dtype=NPBF16)
    Xp[:, :, 1:H + 1, 1:W + 1] = X.astype(NPBF16)
    # w2[(s,i), (t,o)] = weight[o,i,kh,kw] transposed, replicated halves
    wTi = weight.transpose(2, 3, 1, 0).reshape(KS * KS, C, C)  # [t, i, o]
    wTi = wTi.transpose(1, 0, 2).reshape(C, KS * KS * C)       # [i, (t,o)]
    w2 = np.ascontiguousarray(np.concatenate([wTi, wTi], axis=0))
    eye2 = np.tile(np.eye(C, dtype=np.float32), (1, SPC))      # [o', (h,o)]
    in_maps = []
    NW = KS * KS * C
    for c in range(NCORES):
        xs = Xp[c * SPC:(c + 1) * SPC]
        ys = y[c * SPC:(c + 1) * SPC]
        w2xm = np.zeros((2 * C, NW + 132), dtype=np.float32)
        w2xm[:, 0:NW] = w2
        w2xm[:, NW] = CKAIMING * ys.reshape(2 * C)
        w2xm[0:C, NW + 1:NW + 3] = ys.T
        w2xm[0:C, NW + 4:NW + 132] = eye2
        in_maps.append({
            "Xl": np.ascontiguousarray(xs.reshape(SPC * C, HP, WP)),
            "w2x": w2xm,
        })
    return in_maps


def kernel(X, y, weight):
    nc = _get_compiled()
    in_maps = make_in_maps(X, y, weight)
    res = run_bass_kernel_spmd(nc, in_maps, core_ids=list(range(NCORES)))
    outs = [res.results[c]["out"].astype(np.float32).reshape(SPC, C, H, W)
            for c in range(NCORES)]
    return np.concatenate(outs, axis=0)
